# revision 13
# baseline (speedup 1.0000x reference)
"""Multi-head attention (B=4, S=2048, D=1024, H=16) on 8 TRN2 NeuronCores.

Sharding: 8-way over (batch, seq-half). Core c handles batch b=c//2,
query rows sh*1024..sh*1024+1024 (sh=c%2), all 16 heads. K/V are
computed per-batch on both cores of a pair, avoiding any cross-core
collective: the output is a pure concatenation.

Per-core the kv axis is ROTATED by sh*1024 (host-side, consistently for
xt_c / mask / value) so the core's own query columns always sit at
columns 0..1023 of xt_c — attention sums over kv, so any consistent kv
permutation is legal. This removes the separate xt_q input.

v2 schedule: ACT(exp)-bound inner loop with the PE row-tiled.
  - Scores for BOTH heads of a pair are issued back-to-back: head A
    occupies PE rows 0-63 (its dk), head B rows 64-127. Row-disjoint
    matmuls run concurrently in the PE array (per-subarray
    concurrency), so a kv-step's scores cost ~1 matmul of wall time.
  - Each (pair, qh) sub-iteration covers a 512-wide q chunk over all 16
    kv tiles. Scores PSUM tile [128,1024] = [S_A(512) | S_B(512)] so a
    single ACT exp covers both heads.
  - exp on ACT (the only engine with exp); multiplicative 0/1 mask on
    DVE/Pool alternating; PV accumulates [65,512] per head with a
    ones-column in V giving the softmax denominator.
  - V proj, Q/K proj for later pairs, and DMAs are drip-fed into the PE
    between attention matmuls (2 closure-units per kv step).
  - PSUM (8 banks): 2x scores [128,1024] (4) + 2x PV [65,512] (2) +
    proj accum [128,512] (1) + V-proj accum [128,512] (1).

All matmuls in bf16 with fp32 PSUM accumulation.
"""

import sys

if "/opt/trn_rl_repo" not in sys.path:
    sys.path.insert(0, "/opt/trn_rl_repo")

import numpy as np
import ml_dtypes

B, S, D, H = 4, 2048, 1024, 16
DK = D // H  # 64
NCORES = 8
SQ = (B * S) // NCORES  # 1024 query rows per core
NP = H // 2  # 8 head pairs
NDT = D // 128  # 8 d-tiles
NKV = S // 128  # 16 kv tiles
BF16 = ml_dtypes.bfloat16

_CACHE = {}


def _patch_tile_drain():
    """This walrus build rejects >1 sem-wait on the CTRL (drain) struct and
    wide sem-range clears; split the Tile tail-drain's waits and chunk the
    semaphore frees."""
    import concourse.tile as tile
    import concourse.mybir as mybir
    from concourse.vector_clock import ScopedClock

    if getattr(tile.TileContext, "_drain_split_patched", False):
        return

    def _drain_and_barrier(self, tick_clock, wait_clock):
        nc = self.nc
        drain_inst = nc.sync.drain()
        wait_clock.add_sem_waits(
            drain_inst.ins, ScopedClock({None: tick_clock.global_clock})
        )
        si = drain_inst.ins.sync_info
        if si is not None and len(si.on_wait) > 1:
            waits = list(si.on_wait)
            drain_inst.ins.sync_info = mybir.SyncInfo(
                on_wait=waits[:1], on_update=list(si.on_update)
            )
            for w in waits[1:]:
                extra = nc.sync.drain()
                extra.ins.sync_info = mybir.SyncInfo(on_wait=[w], on_update=[])
        nc.all_engine_barrier()
        popped = nc._tile_sem_poison_stack.pop()
        assert popped is self._sem_poison
        sems = sorted(
            self.sems.allocated().values(),
            key=lambda s: s.num if hasattr(s, "num") else s,
        )
        for i in range(0, len(sems), 3):
            nc.clear_and_free_semaphores(sems[i : i + 3])
        nc.all_engine_barrier()

    tile.TileContext._drain_and_barrier = _drain_and_barrier
    tile.TileContext._drain_split_patched = True


def _split_excess_waits(nc, max_waits=1):
    """Walrus (this build) rejects instructions with more than one sem-wait.
    Move overflow waits onto same-engine EventSemaphore instructions inserted
    just before the overloaded instruction (per-engine order preserved)."""
    import concourse.mybir as mybir

    n = 0
    for fn in nc.m.functions:
        for bb in fn.blocks:
            out = []
            changed = False
            for inst in bb.instructions:
                si = getattr(inst, "sync_info", None)
                waits = list(si.on_wait) if si is not None else []
                if len(waits) > max_waits:
                    for w in waits[:-max_waits]:
                        n += 1
                        ev = mybir.InstEventSemaphore(
                            name=f"WSPLIT-{n}", ins=[], outs=[]
                        )
                        ev.engine = inst.engine
                        ev.sync_info = mybir.SyncInfo(on_wait=[w], on_update=[])
                        out.append(ev)
                    inst.sync_info = mybir.SyncInfo(
                        on_wait=waits[-max_waits:], on_update=list(si.on_update)
                    )
                    changed = True
                out.append(inst)
            if changed:
                bb.instructions = out
    return n


def _build():
    """Build the single-core SPMD Bass program (same for all 8 cores)."""
    import concourse.bass as bass
    import concourse.tile as tile
    import concourse.mybir as mybir

    _patch_tile_drain()

    f32 = mybir.dt.float32
    bf16 = mybir.dt.bfloat16
    ACT = mybir.ActivationFunctionType

    nc = bass.Bass("TRN2", target_bir_lowering=False, debug=False)

    # ---- kernel I/O (per-core shards, host-prepped, kv-rotated) ----
    xt_c = nc.dram_tensor("xt_c", [D, S], bf16, kind="ExternalInput").ap()
    # value seq, kv-major: [kv, 128 part, d-tile, 128 col]
    xt_v2 = nc.dram_tensor(
        "xt_v2", [NKV, 128, NDT, 128], bf16, kind="ExternalInput"
    ).ap()
    # mask^T slice, 1.0 = KEEP (multiplicative, applied post-exp)
    mskt = nc.dram_tensor("mskt", [S, SQ], bf16, kind="ExternalInput").ap()
    # wq/wk: [pair, dtile, 128 d, 128 cols(2 heads x 64 dk)]
    wq = nc.dram_tensor("wq", [NP, NDT, 128, 128], bf16, kind="ExternalInput").ap()
    wk = nc.dram_tensor("wk", [NP, NDT, 128, 128], bf16, kind="ExternalInput").ap()
    # wv: [dtile, 128 d, 1024 cols(16 heads x 64)]
    wv = nc.dram_tensor("wv", [NDT, 128, D], bf16, kind="ExternalInput").ap()
    # wot: Wo^T tiled [dtile, 128 din, 1024 dout]
    wot = nc.dram_tensor("wot", [NDT, 128, D], bf16, kind="ExternalInput").ap()
    bq_t = nc.dram_tensor("bq_t", [128, NP], f32, kind="ExternalInput").ap()
    bk_t = nc.dram_tensor("bk_t", [128, NP], f32, kind="ExternalInput").ap()
    bv_t = nc.dram_tensor("bv_t", [DK, H], f32, kind="ExternalInput").ap()
    bo_bc = nc.dram_tensor("bo_bc", [128, D], f32, kind="ExternalInput").ap()
    y = nc.dram_tensor("y", [SQ, D], f32, kind="ExternalOutput").ap()

    with tile.TileContext(nc) as tc:
        with (
            tc.tile_pool(name="persist", bufs=1) as persist,
            # PSUM pools (8 banks of [128, 2KB] total):
            tc.tile_pool(name="pssc", bufs=2, space="PSUM") as pssc,  # 4 banks
            tc.tile_pool(name="pspv", bufs=2, space="PSUM") as pspv,  # 2 banks
            # proj + vproj drip accumulators share one 2-slot pool so
            # back-to-back drip chunks don't serialize on the copy-out
            tc.tile_pool(name="pswk", bufs=2, space="PSUM") as pswk,  # 2 banks
        ):
            # ---- persistent small tensors ----
            bq_sb = persist.tile([128, NP], f32, tag="bq")
            nc.sync.dma_start(bq_sb[:], bq_t[:])
            bk_sb = persist.tile([128, NP], f32, tag="bk")
            nc.sync.dma_start(bk_sb[:], bk_t[:])
            bv_sb = persist.tile([DK, H], f32, tag="bv")
            nc.sync.dma_start(bv_sb[:], bv_t[:])
            bo_sb = persist.tile([128, D], f32, tag="bo")
            nc.sync.dma_start(bo_sb[:], bo_bc[:])

            # V augmented with a ones column per head: [128 kv, 2*65]
            # (col 64/129 = 1.0 -> PV matmul row 64 = softmax denominator)
            vaug_p = [
                persist.tile([128, NKV * 130], bf16, tag=f"va{p}", name=f"va{p}")
                for p in range(NP)
            ]

            def vaug(p, kv):
                return vaug_p[p][:, kv * 130 : (kv + 1) * 130]

            # concat_T: 8 din-tiles [128, SQ]
            concat = [
                persist.tile([128, SQ], bf16, tag=f"cc{p}", name=f"cc{p}")
                for p in range(NP)
            ]

            with (
                tc.tile_pool(name="xtc", bufs=1) as xtcp,
                tc.tile_pool(name="xv", bufs=3) as xvp,
                tc.tile_pool(name="wvp", bufs=1) as wvp,
                tc.tile_pool(name="wqk", bufs=2) as wqkp,
                tc.tile_pool(name="qkt", bufs=3) as qktp,
                tc.tile_pool(name="pexp", bufs=2) as pexp,
                tc.tile_pool(name="pmask", bufs=2) as pmask,
                tc.tile_pool(name="fin", bufs=2) as finp,
                tc.tile_pool(name="wot", bufs=1) as wotp,
                tc.tile_pool(name="outsb", bufs=2) as outp,
                tc.tile_pool(name="dscr", bufs=2, space="DRAM") as dscr,
            ):
                # ---- startup DMAs, priority order ----
                wq0_sb = wqkp.tile([128, NDT, 128], bf16, tag="wq", name="wq0")
                nc.sync.dma_start(wq0_sb[:], wq[0].rearrange("t d c -> d t c"))
                wk0_sb = wqkp.tile([128, NDT, 128], bf16, tag="wk", name="wk0")
                nc.sync.dma_start(wk0_sb[:], wk[0].rearrange("t d c -> d t c"))
                xtc_sb = []
                for d in range(NDT):
                    t = xtcp.tile([128, S], bf16, tag=f"xtc{d}", name=f"xtc{d}")
                    nc.sync.dma_start(t[:], xt_c[d * 128 : (d + 1) * 128, :])
                    xtc_sb.append(t)
                msk_sb = []
                for kv in range(4):
                    t = persist.tile([128, SQ], bf16, tag=f"m{kv}", name=f"m{kv}")
                    nc.sync.dma_start(t[:], mskt[kv * 128 : (kv + 1) * 128, :])
                    msk_sb.append(t)
                wv_sb = []
                for d in range(NDT):
                    t = wvp.tile([128, D], bf16, tag=f"wv{d}", name=f"wv{d}")
                    nc.sync.dma_start(t[:], wv[d, :, :])
                    wv_sb.append(t)
                for kv in range(4, NKV):
                    t = persist.tile([128, SQ], bf16, tag=f"m{kv}", name=f"m{kv}")
                    nc.sync.dma_start(t[:], mskt[kv * 128 : (kv + 1) * 128, :])
                    msk_sb.append(t)

                for p in range(NP):
                    ones_ap = vaug_p[p].rearrange("a (k c) -> a k c", c=65)[
                        :, :, 64:65
                    ]
                    nc.gpsimd.memset(ones_ap, 1.0)

                qt_all = [None] * NP
                kt_all = [None] * NP
                wqk_sb = {0: (wq0_sb, wk0_sb)}

                # ---- emitters (closure-units of ~2 matmuls each) ----
                def proj_steps(p, dma_w=True):
                    """Q/K projection for pair p. Q reads xt_c cols 0..1023
                    (the kv rotation puts this core's queries there)."""
                    state = {}

                    def dma_weights():
                        wq_sb = wqkp.tile(
                            [128, NDT, 128], bf16, tag="wq", name=f"wq{p}"
                        )
                        nc.sync.dma_start(wq_sb[:], wq[p].rearrange("t d c -> d t c"))
                        wk_sb = wqkp.tile(
                            [128, NDT, 128], bf16, tag="wk", name=f"wk{p}"
                        )
                        nc.sync.dma_start(wk_sb[:], wk[p].rearrange("t d c -> d t c"))
                        wqk_sb[p] = (wq_sb, wk_sb)

                    if dma_w:
                        yield dma_weights

                    def alloc_qt():
                        qt_all[p] = qktp.tile([128, SQ], bf16, tag="qt", name=f"qt{p}")
                        kt_all[p] = qktp.tile([128, S], bf16, tag="kt", name=f"kt{p}")

                    yield alloc_qt

                    def chunk(which, ck):
                        cs = slice(ck * 512, (ck + 1) * 512)

                        def mm_a():
                            w_sb = wqk_sb[p][0 if which == "q" else 1]
                            ps = pswk.tile(
                                [128, 512], f32, tag="wk", name=f"pj{which}{p}_{ck}"
                            )
                            state["ps"] = ps
                            for d in range(4):
                                nc.tensor.matmul(
                                    ps[:],
                                    w_sb[:, d, :],
                                    xtc_sb[d][:, cs],
                                    start=(d == 0),
                                    stop=False,
                                )

                        def mm_b():
                            w_sb = wqk_sb[p][0 if which == "q" else 1]
                            ps = state["ps"]
                            for d in range(4, NDT):
                                nc.tensor.matmul(
                                    ps[:],
                                    w_sb[:, d, :],
                                    xtc_sb[d][:, cs],
                                    start=False,
                                    stop=(d == NDT - 1),
                                )
                            dst = qt_all[p] if which == "q" else kt_all[p]
                            bias = bq_sb if which == "q" else bk_sb
                            # gpsimd cannot read PSUM; bias-add on DVE
                            nc.vector.tensor_scalar_add(
                                dst[:, cs], ps[:], bias[:, p : p + 1]
                            )

                        return [mm_a, mm_b]

                    for ck in range(2):  # Q: q cols 0..1023
                        yield from chunk("q", ck)
                    for ck in range(4):  # K: all of S
                        yield from chunk("k", ck)

                def vproj_steps(kv, ch):
                    """V projection for (kv tile, 512-col chunk ch): DMA x
                    chunk, 8 matmuls into [128,512], 4 Pool copies to vaug."""
                    state = {}

                    def dma_xv():
                        t = xvp.tile(
                            [128, NDT, 128], bf16, tag="xv", name=f"xv{kv}_{ch}"
                        )
                        nc.sync.dma_start(t[:], xt_v2[kv])
                        state["xv"] = t

                    yield dma_xv

                    def v_mm(d0):
                        def f():
                            if d0 == 0:
                                state["ps"] = pswk.tile(
                                    [128, 512], f32, tag="wk", name=f"psv{kv}_{ch}"
                                )
                            ps = state["ps"]
                            for d in range(d0, d0 + 2):
                                nc.tensor.matmul(
                                    ps[:],
                                    state["xv"][:, d, :],
                                    wv_sb[d][:, ch * 512 : (ch + 1) * 512],
                                    start=(d == 0),
                                    stop=(d == NDT - 1),
                                )

                        return f

                    for d0 in range(0, NDT, 2):
                        yield v_mm(d0)

                    def v_copy():
                        ps = state["ps"]
                        for pi in range(4):
                            p = ch * 4 + pi
                            dst = vaug(p, kv).rearrange("a (h c) -> a h c", c=65)[
                                :, :, 0:64
                            ]
                            src = ps[:, pi * 128 : (pi + 1) * 128].rearrange(
                                "a (h c) -> a h c", c=64
                            )
                            nc.vector.tensor_copy(dst, src)

                    yield v_copy

                from collections import deque

                work = deque()

                def drain(n):
                    for _ in range(n):
                        if work:
                            work.popleft()()

                # ---- upfront: Q/K proj pair 0, V proj ch0 for kv 0..13 ----
                for step in proj_steps(0, dma_w=False):
                    step()
                for kv in range(14):
                    for step in vproj_steps(kv, 0):
                        step()

                # drip queue, deadline-ordered; proj p+2 is enqueued at
                # pair-p start (its qkt pool slot frees exactly then)
                for kv in (14, 15):
                    work.extend(vproj_steps(kv, 0))
                work.extend(proj_steps(1))
                for kv in range(8):
                    work.extend(vproj_steps(kv, 1))
                work.extend(proj_steps(2))
                for kv in range(8, NKV):
                    work.extend(vproj_steps(kv, 1))

                wot_sb = []

                def dma_wot():
                    for d in range(NDT):
                        t = wotp.tile([128, D], bf16, tag=f"wot{d}", name=f"wot{d}")
                        nc.sync.dma_start(t[:], wot[d, :, :])
                        wot_sb.append(t)

                # ---- attention: pair p, q-half qh, kv 0..15 ----
                mask_eng = [nc.vector, nc.gpsimd]

                def fin_head(p, h, qh, ps_o):
                    """Normalize PV accum [65,512] -> concat; reciprocal of
                    the denominator via DRAM bounce into all DVE lanes."""
                    head = 2 * p + h
                    qs = slice(qh * 512, (qh + 1) * 512)
                    o_sb = finp.tile([65, 512], f32, tag="osb")
                    nc.vector.tensor_copy(o_sb[:], ps_o[:])
                    dsum = dscr.tile([512], f32, tag="dsum")
                    nc.sync.dma_start(
                        dsum.rearrange("(a b) -> a b", a=1), o_sb[64:65, :]
                    )
                    rs = finp.tile([128, 4], f32, tag="rs")
                    nc.sync.dma_start(rs[:], dsum.rearrange("(a b) -> a b", a=128))
                    rr = finp.tile([128, 4], f32, tag="rr")
                    nc.vector.reciprocal(rr[:], rs[:])
                    drec = dscr.tile([512], f32, tag="drec")
                    nc.sync.dma_start(drec.rearrange("(a b) -> a b", a=128), rr[:])
                    rb = finp.tile([64, 512], f32, tag="rb")
                    nc.sync.dma_start(
                        rb[:],
                        drec.rearrange("(a b) -> a b", a=1).partition_broadcast(64),
                    )
                    tmp = finp.tile([64, 512], f32, tag="tmp")
                    nc.gpsimd.tensor_mul(tmp[:], rb[:], o_sb[0:64, :])
                    nc.gpsimd.tensor_scalar_add(
                        concat[p][h * 64 : (h + 1) * 64, qs],
                        tmp[:],
                        bv_sb[:, head : head + 1],
                    )

                pending_fin = deque()
                for p in range(NP):
                    if 1 <= p and p + 2 < NP:
                        work.extend(proj_steps(p + 2))
                    if p == 5:
                        work.append(dma_wot)
                    qt, kt = qt_all[p], kt_all[p]
                    for qh in range(2):
                        qs = slice(qh * 512, (qh + 1) * 512)
                        ps_oA = pspv.tile([65, 512], f32, tag="po", name=f"poA{p}_{qh}")
                        ps_oB = pspv.tile([65, 512], f32, tag="po", name=f"poB{p}_{qh}")
                        prev_pm = None
                        for kv in range(NKV):
                            kvs = slice(kv * 128, (kv + 1) * 128)
                            ps_s = pssc.tile(
                                [128, 1024], f32, tag="ps", name=f"s{p}_{qh}_{kv}"
                            )
                            # row-tiled scores: head A rows 0-63, B rows 64-127
                            nc.tensor.matmul(
                                ps_s[:, 0:512],
                                kt[0:64, kvs],
                                qt[0:64, qs],
                                start=True,
                                stop=True,
                            )
                            nc.tensor.matmul(
                                ps_s[:, 512:1024],
                                kt[64:128, kvs],
                                qt[64:128, qs],
                                start=True,
                                stop=True,
                            )
                            if prev_pm is not None:
                                # PV for kv-1 (both heads), one behind
                                nc.tensor.matmul(
                                    ps_oA[:],
                                    vaug(p, kv - 1)[:, 0:65],
                                    prev_pm[:, 0:512],
                                    start=(kv == 1),
                                    stop=False,
                                )
                                nc.tensor.matmul(
                                    ps_oB[:],
                                    vaug(p, kv - 1)[:, 65:130],
                                    prev_pm[:, 512:1024],
                                    start=(kv == 1),
                                    stop=False,
                                )
                            drain(2)
                            if pending_fin and kv == 1:
                                fin_head(*pending_fin.popleft())
                            if pending_fin and kv == 2:
                                fin_head(*pending_fin.popleft())
                            pe_t = pexp.tile([128, 1024], bf16, tag="pe")
                            nc.scalar.activation(
                                pe_t[:], ps_s[:], ACT.Exp, scale=0.125
                            )
                            pm_t = pmask.tile([128, 1024], bf16, tag="pm")
                            me = mask_eng[kv % 2]
                            me.tensor_mul(
                                pm_t[:, 0:512], pe_t[:, 0:512], msk_sb[kv][:, qs]
                            )
                            me.tensor_mul(
                                pm_t[:, 512:1024], pe_t[:, 512:1024], msk_sb[kv][:, qs]
                            )
                            prev_pm = pm_t
                        # last PV (kv = 15)
                        nc.tensor.matmul(
                            ps_oA[:],
                            vaug(p, NKV - 1)[:, 0:65],
                            prev_pm[:, 0:512],
                            start=False,
                            stop=True,
                        )
                        nc.tensor.matmul(
                            ps_oB[:],
                            vaug(p, NKV - 1)[:, 65:130],
                            prev_pm[:, 512:1024],
                            start=False,
                            stop=True,
                        )
                        pending_fin.append((p, 0, qh, ps_oA))
                        pending_fin.append((p, 1, qh, ps_oB))
                while pending_fin:
                    fin_head(*pending_fin.popleft())
                while work:
                    work.popleft()()

                # ---- output projection ----
                for qt_i in range(SQ // 128):
                    qs = slice(qt_i * 128, (qt_i + 1) * 128)
                    for ch in range(2):
                        chs = slice(ch * 512, (ch + 1) * 512)
                        ps_f = pssc.tile(
                            [128, 1024], f32, tag="ps", name=f"of{qt_i}_{ch}"
                        )
                        for d in range(NDT):
                            nc.tensor.matmul(
                                ps_f[:, 0:512],
                                concat[d][:, qs],
                                wot_sb[d][:, chs],
                                start=(d == 0),
                                stop=(d == NDT - 1),
                            )
                        out_sb = outp.tile([128, 512], f32, tag="out")
                        nc.vector.tensor_add(out_sb[:], ps_f[:, 0:512], bo_sb[:, chs])
                        nc.sync.dma_start(y[qs, chs], out_sb[:])

    _split_excess_waits(nc, max_waits=1)
    return nc


def _prep_inputs(context_sequence, value_sequence, mask, Wq, bq, Wk, bk, Wv, bv, Wo, bo):
    """Host-side shard prep: slice/transpose/cast per core, with the kv
    axis rotated by sh*SQ so queries are at columns 0..SQ-1."""
    ctx = np.asarray(context_sequence, dtype=np.float32)
    val = np.asarray(value_sequence, dtype=np.float32)
    mask = np.asarray(mask)
    Wq = np.asarray(Wq, dtype=np.float32)
    Wk = np.asarray(Wk, dtype=np.float32)
    Wv = np.asarray(Wv, dtype=np.float32)
    Wo = np.asarray(Wo, dtype=np.float32)
    bq = np.asarray(bq, dtype=np.float32)
    bk = np.asarray(bk, dtype=np.float32)
    bv = np.asarray(bv, dtype=np.float32)
    bo = np.asarray(bo, dtype=np.float32)

    def wtile(W):  # [H, D, DK] -> [NP, NDT, 128, 128]
        Wf = W.transpose(1, 0, 2).reshape(D, D)  # [d, h*dk]
        return np.ascontiguousarray(
            Wf.reshape(NDT, 128, NP, 128).transpose(2, 0, 1, 3)
        ).astype(BF16)

    wq_t = wtile(Wq)
    wk_t = wtile(Wk)
    wv_t = np.ascontiguousarray(
        Wv.transpose(1, 0, 2).reshape(D, D).reshape(NDT, 128, D)
    ).astype(BF16)
    wot_t = np.ascontiguousarray(Wo.T.reshape(NDT, 128, D)).astype(BF16)
    bq_t = np.ascontiguousarray(bq.reshape(NP, 128).T)  # [128, NP]
    bk_t = np.ascontiguousarray(bk.reshape(NP, 128).T)
    bv_t = np.ascontiguousarray(bv.reshape(H, DK).T)  # [DK, H]
    bo_bc = np.ascontiguousarray(np.broadcast_to(bo[None, :], (128, D)))

    in_maps = []
    for c in range(NCORES):
        b, sh = c // 2, c % 2
        roll = -sh * SQ  # rotate kv so this core's queries sit at cols 0..SQ-1
        xt = np.ascontiguousarray(
            np.roll(ctx[b].T, roll, axis=1)
        ).astype(BF16)  # [D, S], kv-rotated
        xtv = np.roll(val[b].T, roll, axis=1).astype(BF16)  # [D, S]
        xtv2 = np.ascontiguousarray(
            xtv.reshape(NDT, 128, NKV, 128).transpose(2, 1, 0, 3)
        )  # [kv, p, d, c]
        mskt = np.ascontiguousarray(
            np.roll(
                (mask[sh * SQ : (sh + 1) * SQ, :] == 0).T.astype(BF16),
                roll,
                axis=0,
            )
        )  # [S, SQ], 1.0 = keep, kv-rotated
        in_maps.append(
            {
                "xt_c": xt,
                "xt_v2": xtv2,
                "mskt": mskt,
                "wq": wq_t,
                "wk": wk_t,
                "wv": wv_t,
                "wot": wot_t,
                "bq_t": bq_t,
                "bk_t": bk_t,
                "bv_t": bv_t,
                "bo_bc": bo_bc,
            }
        )
    return in_maps


def _execute(inputs, trace=False):
    from concourse.bass_utils import run_bass_kernel_spmd

    if "nc" not in _CACHE:
        _CACHE["nc"] = _build()
    nc = _CACHE["nc"]
    in_maps = _prep_inputs(**inputs)
    res = run_bass_kernel_spmd(nc, in_maps, list(range(NCORES)), trace=trace)
    out = np.empty((B, S, D), dtype=np.float32)
    for c in range(NCORES):
        b, sh = c // 2, c % 2
        out[b, sh * SQ : (sh + 1) * SQ, :] = res.results[c]["y"]
    return out, res.exec_time_ns


def kernel(**inputs):
    out, _ = _execute(inputs, trace=False)
    return out


# revision 21
# speedup vs baseline: 1.7485x; 1.7485x over previous
"""Multi-head attention (B=4, S=2048, D=1024, H=16) on 8 TRN2 NeuronCores.

Sharding: 8-way over (batch, seq-half). Core c handles batch b=c//2,
query rows sh*1024..sh*1024+1024 (sh=c%2), all 16 heads. K/V are
computed per-batch on both cores of a pair, avoiding any cross-core
collective: the output is a pure concatenation.

Per-core the kv axis is ROTATED by sh*1024 (host-side, consistently for
xt_c / mask / value) so the core's own query columns always sit at
columns 0..1023 of xt_c — attention sums over kv, so any consistent kv
permutation is legal. This removes the separate xt_q input.

v2 schedule: ACT(exp)-bound inner loop with the PE row-tiled.
  - Scores for BOTH heads of a pair are issued back-to-back: head A
    occupies PE rows 0-63 (its dk), head B rows 64-127. Row-disjoint
    matmuls run concurrently in the PE array (per-subarray
    concurrency), so a kv-step's scores cost ~1 matmul of wall time.
  - Each (pair, qh) sub-iteration covers a 512-wide q chunk over all 16
    kv tiles. Scores PSUM tile [128,1024] = [S_A(512) | S_B(512)] so a
    single ACT exp covers both heads.
  - exp on ACT (the only engine with exp); multiplicative 0/1 mask on
    DVE/Pool alternating; PV accumulates [65,512] per head with a
    ones-column in V giving the softmax denominator.
  - V proj, Q/K proj for later pairs, and DMAs are drip-fed into the PE
    between attention matmuls (2 closure-units per kv step).
  - PSUM (8 banks): 2x scores [128,1024] (4) + 2x PV [65,512] (2) +
    proj accum [128,512] (1) + V-proj accum [128,512] (1).

All matmuls in bf16 with fp32 PSUM accumulation.
"""

import sys

if "/opt/trn_rl_repo" not in sys.path:
    sys.path.insert(0, "/opt/trn_rl_repo")

import numpy as np
import ml_dtypes

B, S, D, H = 4, 2048, 1024, 16
DK = D // H  # 64
NCORES = 8
SQ = (B * S) // NCORES  # 1024 query rows per core
NP = H // 2  # 8 head pairs
NDT = D // 128  # 8 d-tiles
NKV = S // 128  # 16 kv tiles
BF16 = ml_dtypes.bfloat16

_CACHE = {}


def _patch_tile_drain():
    """This walrus build rejects >1 sem-wait on the CTRL (drain) struct and
    wide sem-range clears; split the Tile tail-drain's waits and chunk the
    semaphore frees."""
    import concourse.tile as tile
    import concourse.mybir as mybir
    from concourse.vector_clock import ScopedClock

    if getattr(tile.TileContext, "_drain_split_patched", False):
        return

    def _drain_and_barrier(self, tick_clock, wait_clock):
        nc = self.nc
        drain_inst = nc.sync.drain()
        wait_clock.add_sem_waits(
            drain_inst.ins, ScopedClock({None: tick_clock.global_clock})
        )
        si = drain_inst.ins.sync_info
        if si is not None and len(si.on_wait) > 1:
            waits = list(si.on_wait)
            drain_inst.ins.sync_info = mybir.SyncInfo(
                on_wait=waits[:1], on_update=list(si.on_update)
            )
            for w in waits[1:]:
                extra = nc.sync.drain()
                extra.ins.sync_info = mybir.SyncInfo(on_wait=[w], on_update=[])
        nc.all_engine_barrier()
        popped = nc._tile_sem_poison_stack.pop()
        assert popped is self._sem_poison
        sems = sorted(
            self.sems.allocated().values(),
            key=lambda s: s.num if hasattr(s, "num") else s,
        )
        for i in range(0, len(sems), 3):
            nc.clear_and_free_semaphores(sems[i : i + 3])
        nc.all_engine_barrier()

    tile.TileContext._drain_and_barrier = _drain_and_barrier
    tile.TileContext._drain_split_patched = True


def _split_excess_waits(nc, max_waits=1):
    """Walrus (this build) rejects instructions with more than one sem-wait.
    Move overflow waits onto same-engine EventSemaphore instructions inserted
    just before the overloaded instruction (per-engine order preserved)."""
    import concourse.mybir as mybir

    n = 0
    for fn in nc.m.functions:
        for bb in fn.blocks:
            out = []
            changed = False
            for inst in bb.instructions:
                si = getattr(inst, "sync_info", None)
                waits = list(si.on_wait) if si is not None else []
                if len(waits) > max_waits:
                    for w in waits[:-max_waits]:
                        n += 1
                        ev = mybir.InstEventSemaphore(
                            name=f"WSPLIT-{n}", ins=[], outs=[]
                        )
                        ev.engine = inst.engine
                        ev.sync_info = mybir.SyncInfo(on_wait=[w], on_update=[])
                        out.append(ev)
                    inst.sync_info = mybir.SyncInfo(
                        on_wait=waits[-max_waits:], on_update=list(si.on_update)
                    )
                    changed = True
                out.append(inst)
            if changed:
                bb.instructions = out
    return n


def _build():
    """Build the single-core SPMD Bass program (same for all 8 cores)."""
    import concourse.bass as bass
    import concourse.tile as tile
    import concourse.mybir as mybir

    _patch_tile_drain()

    f32 = mybir.dt.float32
    bf16 = mybir.dt.bfloat16
    ACT = mybir.ActivationFunctionType

    nc = bass.Bass("TRN2", target_bir_lowering=False, debug=False)

    # ---- kernel I/O (per-core shards, host-prepped, kv-rotated) ----
    xt_c = nc.dram_tensor("xt_c", [D, S], bf16, kind="ExternalInput").ap()
    # value seq, kv-major: [kv, 128 part, d-tile, 128 col]
    xt_v2 = nc.dram_tensor(
        "xt_v2", [NKV, 128, NDT, 128], bf16, kind="ExternalInput"
    ).ap()
    # mask^T slice, 1.0 = KEEP (multiplicative, applied post-exp)
    mskt = nc.dram_tensor("mskt", [S, SQ], bf16, kind="ExternalInput").ap()
    # wq/wk: [pair, dtile, 128 d, 128 cols(2 heads x 64 dk)]
    wq = nc.dram_tensor("wq", [NP, NDT, 128, 128], bf16, kind="ExternalInput").ap()
    wk = nc.dram_tensor("wk", [NP, NDT, 128, 128], bf16, kind="ExternalInput").ap()
    # wv: [dtile, 128 d, 1024 cols(16 heads x 64)]
    wv = nc.dram_tensor("wv", [NDT, 128, D], bf16, kind="ExternalInput").ap()
    # wot: Wo^T tiled [dtile, 128 din, 1024 dout]
    wot = nc.dram_tensor("wot", [NDT, 128, D], bf16, kind="ExternalInput").ap()
    bq_t = nc.dram_tensor("bq_t", [128, NP], f32, kind="ExternalInput").ap()
    bk_t = nc.dram_tensor("bk_t", [128, NP], f32, kind="ExternalInput").ap()
    # bv broadcast to all kv partitions, cols = h*64+c (folded into vaug:
    # sum_kv p*(V+bv) / sum_kv p == PV/sum + bv exactly)
    bv_bc = nc.dram_tensor("bv_bc", [128, D], f32, kind="ExternalInput").ap()
    bo_bc = nc.dram_tensor("bo_bc", [128, D], f32, kind="ExternalInput").ap()
    y = nc.dram_tensor("y", [SQ, D], f32, kind="ExternalOutput").ap()

    with tile.TileContext(nc) as tc:
        with (
            tc.tile_pool(name="persist", bufs=1) as persist,
            # PSUM pools (8 banks of [128, 2KB] total):
            tc.tile_pool(name="pssc", bufs=2, space="PSUM") as pssc,  # 4 banks
            tc.tile_pool(name="pspv", bufs=2, space="PSUM") as pspv,  # 2 banks
            # proj + vproj drip accumulators share one 2-slot pool so
            # back-to-back drip chunks don't serialize on the copy-out
            tc.tile_pool(name="pswk", bufs=2, space="PSUM") as pswk,  # 2 banks
        ):
            # ---- persistent small tensors ----
            bq_sb = persist.tile([128, NP], f32, tag="bq")
            nc.sync.dma_start(bq_sb[:], bq_t[:])
            bk_sb = persist.tile([128, NP], f32, tag="bk")
            nc.sync.dma_start(bk_sb[:], bk_t[:])
            bv_sb = persist.tile([128, D], f32, tag="bv")
            nc.sync.dma_start(bv_sb[:], bv_bc[:])
            bo_sb = persist.tile([128, D], f32, tag="bo")
            nc.sync.dma_start(bo_sb[:], bo_bc[:])

            # V augmented with a ones column per head: [128 kv, 2*65]
            # (col 64/129 = 1.0 -> PV matmul row 64 = softmax denominator)
            vaug_p = [
                persist.tile([128, NKV * 130], bf16, tag=f"va{p}", name=f"va{p}")
                for p in range(NP)
            ]

            def vaug(p, kv):
                return vaug_p[p][:, kv * 130 : (kv + 1) * 130]

            # concat_T: 8 din-tiles [128, SQ]
            concat = [
                persist.tile([128, SQ], bf16, tag=f"cc{p}", name=f"cc{p}")
                for p in range(NP)
            ]

            with (
                tc.tile_pool(name="xtc", bufs=1) as xtcp,
                tc.tile_pool(name="xv", bufs=3) as xvp,
                tc.tile_pool(name="wvp", bufs=1) as wvp,
                tc.tile_pool(name="wqk", bufs=2) as wqkp,
                tc.tile_pool(name="qkt", bufs=3) as qktp,
                tc.tile_pool(name="pexp", bufs=2) as pexp,
                tc.tile_pool(name="pmask", bufs=2) as pmask,
                tc.tile_pool(name="fin", bufs=2) as finp,
                tc.tile_pool(name="wot", bufs=1) as wotp,
                tc.tile_pool(name="outsb", bufs=2) as outp,
                tc.tile_pool(name="dscr", bufs=2, space="DRAM") as dscr,
            ):
                # ---- startup DMAs, priority order ----
                wq0_sb = wqkp.tile([128, NDT, 128], bf16, tag="wq", name="wq0")
                nc.sync.dma_start(wq0_sb[:], wq[0].rearrange("t d c -> d t c"))
                wk0_sb = wqkp.tile([128, NDT, 128], bf16, tag="wk", name="wk0")
                nc.sync.dma_start(wk0_sb[:], wk[0].rearrange("t d c -> d t c"))
                xtc_sb = []
                for d in range(NDT):
                    t = xtcp.tile([128, S], bf16, tag=f"xtc{d}", name=f"xtc{d}")
                    nc.sync.dma_start(t[:], xt_c[d * 128 : (d + 1) * 128, :])
                    xtc_sb.append(t)
                msk_sb = []
                for kv in range(4):
                    t = persist.tile([128, SQ], bf16, tag=f"m{kv}", name=f"m{kv}")
                    nc.sync.dma_start(t[:], mskt[kv * 128 : (kv + 1) * 128, :])
                    msk_sb.append(t)
                wv_sb = []
                for d in range(NDT):
                    t = wvp.tile([128, D], bf16, tag=f"wv{d}", name=f"wv{d}")
                    nc.sync.dma_start(t[:], wv[d, :, :])
                    wv_sb.append(t)
                for kv in range(4, NKV):
                    t = persist.tile([128, SQ], bf16, tag=f"m{kv}", name=f"m{kv}")
                    nc.sync.dma_start(t[:], mskt[kv * 128 : (kv + 1) * 128, :])
                    msk_sb.append(t)

                for p in range(NP):
                    ones_ap = vaug_p[p].rearrange("a (k c) -> a k c", c=65)[
                        :, :, 64:65
                    ]
                    nc.gpsimd.memset(ones_ap, 1.0)

                qt_all = [None] * NP
                kt_all = [None] * NP
                wqk_sb = {0: (wq0_sb, wk0_sb)}

                # ---- emitters (closure-units of ~2 matmuls each) ----
                def proj_steps(p, dma_w=True):
                    """Q/K projection for pair p. Q reads xt_c cols 0..1023
                    (the kv rotation puts this core's queries there)."""
                    state = {}

                    def dma_weights():
                        wq_sb = wqkp.tile(
                            [128, NDT, 128], bf16, tag="wq", name=f"wq{p}"
                        )
                        nc.sync.dma_start(wq_sb[:], wq[p].rearrange("t d c -> d t c"))
                        wk_sb = wqkp.tile(
                            [128, NDT, 128], bf16, tag="wk", name=f"wk{p}"
                        )
                        nc.sync.dma_start(wk_sb[:], wk[p].rearrange("t d c -> d t c"))
                        wqk_sb[p] = (wq_sb, wk_sb)

                    if dma_w:
                        yield dma_weights

                    def alloc_qt():
                        qt_all[p] = qktp.tile([128, SQ], bf16, tag="qt", name=f"qt{p}")
                        kt_all[p] = qktp.tile([128, S], bf16, tag="kt", name=f"kt{p}")

                    yield alloc_qt

                    def chunk(which, ck):
                        cs = slice(ck * 512, (ck + 1) * 512)

                        def mm_a():
                            w_sb = wqk_sb[p][0 if which == "q" else 1]
                            ps = pswk.tile(
                                [128, 512], f32, tag="wk", name=f"pj{which}{p}_{ck}"
                            )
                            state["ps"] = ps
                            for d in range(4):
                                nc.tensor.matmul(
                                    ps[:],
                                    w_sb[:, d, :],
                                    xtc_sb[d][:, cs],
                                    start=(d == 0),
                                    stop=False,
                                )

                        def mm_b():
                            w_sb = wqk_sb[p][0 if which == "q" else 1]
                            ps = state["ps"]
                            for d in range(4, NDT):
                                nc.tensor.matmul(
                                    ps[:],
                                    w_sb[:, d, :],
                                    xtc_sb[d][:, cs],
                                    start=False,
                                    stop=(d == NDT - 1),
                                )
                            dst = qt_all[p] if which == "q" else kt_all[p]
                            bias = bq_sb if which == "q" else bk_sb
                            # gpsimd cannot read PSUM; bias-add on DVE
                            nc.vector.tensor_scalar_add(
                                dst[:, cs], ps[:], bias[:, p : p + 1]
                            )

                        return [mm_a, mm_b]

                    for ck in range(2):  # Q: q cols 0..1023
                        yield from chunk("q", ck)
                    for ck in range(4):  # K: all of S
                        yield from chunk("k", ck)

                def vproj_steps(kv, ch):
                    """V projection for (kv tile, 512-col chunk ch): DMA x
                    chunk, 8 matmuls into [128,512], 4 Pool copies to vaug."""
                    state = {}

                    def dma_xv():
                        t = xvp.tile(
                            [128, NDT, 128], bf16, tag="xv", name=f"xv{kv}_{ch}"
                        )
                        nc.sync.dma_start(t[:], xt_v2[kv])
                        state["xv"] = t

                    yield dma_xv

                    def v_mm(d0):
                        def f():
                            if d0 == 0:
                                state["ps"] = pswk.tile(
                                    [128, 512], f32, tag="wk", name=f"psv{kv}_{ch}"
                                )
                            ps = state["ps"]
                            for d in range(d0, d0 + 2):
                                nc.tensor.matmul(
                                    ps[:],
                                    state["xv"][:, d, :],
                                    wv_sb[d][:, ch * 512 : (ch + 1) * 512],
                                    start=(d == 0),
                                    stop=(d == NDT - 1),
                                )

                        return f

                    for d0 in range(0, NDT, 2):
                        yield v_mm(d0)

                    def v_copy():
                        ps = state["ps"]
                        for pi in range(4):
                            p = ch * 4 + pi
                            dst = vaug(p, kv).rearrange("a (h c) -> a h c", c=65)[
                                :, :, 0:64
                            ]
                            src = ps[:, pi * 128 : (pi + 1) * 128].rearrange(
                                "a (h c) -> a h c", c=64
                            )
                            bvs = bv_sb[:, p * 128 : (p + 1) * 128].rearrange(
                                "a (h c) -> a h c", c=64
                            )
                            nc.vector.tensor_add(dst, src, bvs)

                    yield v_copy

                from collections import deque

                work = deque()

                def drain(n):
                    for _ in range(n):
                        if work:
                            work.popleft()()

                # ---- upfront: Q/K proj pair 0, V proj ch0 for kv 0..13 ----
                for step in proj_steps(0, dma_w=False):
                    step()
                for kv in range(14):
                    for step in vproj_steps(kv, 0):
                        step()

                # drip queue, deadline-ordered; proj p+2 is enqueued at
                # pair-p start (its qkt pool slot frees exactly then)
                for kv in (14, 15):
                    work.extend(vproj_steps(kv, 0))
                work.extend(proj_steps(1))
                for kv in range(8):
                    work.extend(vproj_steps(kv, 1))
                work.extend(proj_steps(2))
                for kv in range(8, NKV):
                    work.extend(vproj_steps(kv, 1))

                wot_sb = []

                def dma_wot():
                    for d in range(NDT):
                        t = wotp.tile([128, D], bf16, tag=f"wot{d}", name=f"wot{d}")
                        nc.sync.dma_start(t[:], wot[d, :, :])
                        wot_sb.append(t)

                # ---- attention: pair p, q-half qh, kv 0..15 ----
                def fin_head(p, h, qh, ps_o):
                    """Normalize PV accum [65,512] -> concat; reciprocal of
                    the denominator via DRAM bounce into all DVE lanes."""
                    qs = slice(qh * 512, (qh + 1) * 512)
                    o_sb = finp.tile([65, 512], f32, tag="osb")
                    nc.vector.tensor_copy(o_sb[:], ps_o[:])
                    dsum = dscr.tile([512], f32, tag="dsum")
                    nc.sync.dma_start(
                        dsum.rearrange("(a b) -> a b", a=1), o_sb[64:65, :]
                    )
                    rs = finp.tile([128, 4], f32, tag="rs")
                    nc.sync.dma_start(rs[:], dsum.rearrange("(a b) -> a b", a=128))
                    rr = finp.tile([128, 4], f32, tag="rr")
                    nc.vector.reciprocal(rr[:], rs[:])
                    drec = dscr.tile([512], f32, tag="drec")
                    nc.sync.dma_start(drec.rearrange("(a b) -> a b", a=128), rr[:])
                    rb = finp.tile([64, 512], f32, tag="rb")
                    nc.sync.dma_start(
                        rb[:],
                        drec.rearrange("(a b) -> a b", a=1).partition_broadcast(64),
                    )
                    nc.vector.tensor_mul(
                        concat[p][h * 64 : (h + 1) * 64, qs], rb[:], o_sb[0:64, :]
                    )

                pending_fin = deque()
                for p in range(NP):
                    if 1 <= p and p + 2 < NP:
                        work.extend(proj_steps(p + 2))
                    if p == 5:
                        work.append(dma_wot)
                    qt, kt = qt_all[p], kt_all[p]
                    for qh in range(2):
                        qs = slice(qh * 512, (qh + 1) * 512)
                        ps_oA = pspv.tile([65, 512], f32, tag="po", name=f"poA{p}_{qh}")
                        ps_oB = pspv.tile([65, 512], f32, tag="po", name=f"poB{p}_{qh}")
                        prev_pm = None
                        for kv in range(NKV):
                            kvs = slice(kv * 128, (kv + 1) * 128)
                            ps_s = pssc.tile(
                                [128, 1024], f32, tag="ps", name=f"s{p}_{qh}_{kv}"
                            )
                            # row-tiled scores: head A rows 0-63, B rows 64-127
                            nc.tensor.matmul(
                                ps_s[:, 0:512],
                                kt[0:64, kvs],
                                qt[0:64, qs],
                                start=True,
                                stop=True,
                            )
                            nc.tensor.matmul(
                                ps_s[:, 512:1024],
                                kt[64:128, kvs],
                                qt[64:128, qs],
                                start=True,
                                stop=True,
                            )
                            if prev_pm is not None:
                                # PV for kv-1 (both heads), one behind
                                nc.tensor.matmul(
                                    ps_oA[:],
                                    vaug(p, kv - 1)[:, 0:65],
                                    prev_pm[:, 0:512],
                                    start=(kv == 1),
                                    stop=False,
                                )
                                nc.tensor.matmul(
                                    ps_oB[:],
                                    vaug(p, kv - 1)[:, 65:130],
                                    prev_pm[:, 512:1024],
                                    start=(kv == 1),
                                    stop=False,
                                )
                            drain(2)
                            if pending_fin and kv == 1:
                                fin_head(*pending_fin.popleft())
                            if pending_fin and kv == 2:
                                fin_head(*pending_fin.popleft())
                            pe_t = pexp.tile([128, 1024], bf16, tag="pe")
                            nc.scalar.activation(
                                pe_t[:], ps_s[:], ACT.Exp, scale=0.125
                            )
                            pm_t = pmask.tile([128, 1024], bf16, tag="pm")
                            mb = (
                                msk_sb[kv][:, qs]
                                .rearrange("p (o c) -> p o c", o=1)
                                .broadcast_to([128, 2, 512])
                            )
                            nc.vector.tensor_mul(
                                pm_t.rearrange("p (o c) -> p o c", c=512),
                                pe_t.rearrange("p (o c) -> p o c", c=512),
                                mb,
                            )
                            prev_pm = pm_t
                        # last PV (kv = 15)
                        nc.tensor.matmul(
                            ps_oA[:],
                            vaug(p, NKV - 1)[:, 0:65],
                            prev_pm[:, 0:512],
                            start=False,
                            stop=True,
                        )
                        nc.tensor.matmul(
                            ps_oB[:],
                            vaug(p, NKV - 1)[:, 65:130],
                            prev_pm[:, 512:1024],
                            start=False,
                            stop=True,
                        )
                        pending_fin.append((p, 0, qh, ps_oA))
                        pending_fin.append((p, 1, qh, ps_oB))
                while pending_fin:
                    fin_head(*pending_fin.popleft())
                while work:
                    work.popleft()()

                # ---- output projection ----
                for qt_i in range(SQ // 128):
                    qs = slice(qt_i * 128, (qt_i + 1) * 128)
                    for ch in range(2):
                        chs = slice(ch * 512, (ch + 1) * 512)
                        ps_f = pssc.tile(
                            [128, 1024], f32, tag="ps", name=f"of{qt_i}_{ch}"
                        )
                        for d in range(NDT):
                            nc.tensor.matmul(
                                ps_f[:, 0:512],
                                concat[d][:, qs],
                                wot_sb[d][:, chs],
                                start=(d == 0),
                                stop=(d == NDT - 1),
                            )
                        out_sb = outp.tile([128, 512], f32, tag="out")
                        nc.vector.tensor_add(out_sb[:], ps_f[:, 0:512], bo_sb[:, chs])
                        nc.sync.dma_start(y[qs, chs], out_sb[:])

    _split_excess_waits(nc, max_waits=1)
    return nc


def _prep_inputs(context_sequence, value_sequence, mask, Wq, bq, Wk, bk, Wv, bv, Wo, bo):
    """Host-side shard prep: slice/transpose/cast per core, with the kv
    axis rotated by sh*SQ so queries are at columns 0..SQ-1."""
    ctx = np.asarray(context_sequence, dtype=np.float32)
    val = np.asarray(value_sequence, dtype=np.float32)
    mask = np.asarray(mask)
    Wq = np.asarray(Wq, dtype=np.float32)
    Wk = np.asarray(Wk, dtype=np.float32)
    Wv = np.asarray(Wv, dtype=np.float32)
    Wo = np.asarray(Wo, dtype=np.float32)
    bq = np.asarray(bq, dtype=np.float32)
    bk = np.asarray(bk, dtype=np.float32)
    bv = np.asarray(bv, dtype=np.float32)
    bo = np.asarray(bo, dtype=np.float32)

    def wtile(W):  # [H, D, DK] -> [NP, NDT, 128, 128]
        Wf = W.transpose(1, 0, 2).reshape(D, D)  # [d, h*dk]
        return np.ascontiguousarray(
            Wf.reshape(NDT, 128, NP, 128).transpose(2, 0, 1, 3)
        ).astype(BF16)

    wq_t = wtile(Wq)
    wk_t = wtile(Wk)
    wv_t = np.ascontiguousarray(
        Wv.transpose(1, 0, 2).reshape(D, D).reshape(NDT, 128, D)
    ).astype(BF16)
    wot_t = np.ascontiguousarray(Wo.T.reshape(NDT, 128, D)).astype(BF16)
    bq_t = np.ascontiguousarray(bq.reshape(NP, 128).T)  # [128, NP]
    bk_t = np.ascontiguousarray(bk.reshape(NP, 128).T)
    bv_bc = np.ascontiguousarray(
        np.broadcast_to(bv.reshape(D)[None, :], (128, D))
    )  # [128, D], cols h*64+c
    bo_bc = np.ascontiguousarray(np.broadcast_to(bo[None, :], (128, D)))

    in_maps = []
    for c in range(NCORES):
        b, sh = c // 2, c % 2
        roll = -sh * SQ  # rotate kv so this core's queries sit at cols 0..SQ-1
        xt = np.ascontiguousarray(
            np.roll(ctx[b].T, roll, axis=1)
        ).astype(BF16)  # [D, S], kv-rotated
        xtv = np.roll(val[b].T, roll, axis=1).astype(BF16)  # [D, S]
        xtv2 = np.ascontiguousarray(
            xtv.reshape(NDT, 128, NKV, 128).transpose(2, 1, 0, 3)
        )  # [kv, p, d, c]
        mskt = np.ascontiguousarray(
            np.roll(
                (mask[sh * SQ : (sh + 1) * SQ, :] == 0).T.astype(BF16),
                roll,
                axis=0,
            )
        )  # [S, SQ], 1.0 = keep, kv-rotated
        in_maps.append(
            {
                "xt_c": xt,
                "xt_v2": xtv2,
                "mskt": mskt,
                "wq": wq_t,
                "wk": wk_t,
                "wv": wv_t,
                "wot": wot_t,
                "bq_t": bq_t,
                "bk_t": bk_t,
                "bv_bc": bv_bc,
                "bo_bc": bo_bc,
            }
        )
    return in_maps


def _execute(inputs, trace=False):
    from concourse.bass_utils import run_bass_kernel_spmd

    if "nc" not in _CACHE:
        _CACHE["nc"] = _build()
    nc = _CACHE["nc"]
    in_maps = _prep_inputs(**inputs)
    res = run_bass_kernel_spmd(nc, in_maps, list(range(NCORES)), trace=trace)
    out = np.empty((B, S, D), dtype=np.float32)
    for c in range(NCORES):
        b, sh = c // 2, c % 2
        out[b, sh * SQ : (sh + 1) * SQ, :] = res.results[c]["y"]
    return out, res.exec_time_ns


def kernel(**inputs):
    out, _ = _execute(inputs, trace=False)
    return out


# revision 26
# speedup vs baseline: 1.8263x; 1.0445x over previous
"""Multi-head attention (B=4, S=2048, D=1024, H=16) on 8 TRN2 NeuronCores.

Sharding: 8-way over (batch, seq-half). Core c handles batch b=c//2,
query rows sh*1024..sh*1024+1024 (sh=c%2), all 16 heads. K/V are
computed per-batch on both cores of a pair, avoiding any cross-core
collective: the output is a pure concatenation.

Per-core the kv axis is ROTATED by sh*1024 (host-side, consistently for
xt_c / mask / value) so the core's own query columns always sit at
columns 0..1023 of xt_c — attention sums over kv, so any consistent kv
permutation is legal. This removes the separate xt_q input.

v2 schedule: ACT(exp)-bound inner loop with the PE row-tiled.
  - Scores for BOTH heads of a pair are issued back-to-back: head A
    occupies PE rows 0-63 (its dk), head B rows 64-127. Row-disjoint
    matmuls run concurrently in the PE array (per-subarray
    concurrency), so a kv-step's scores cost ~1 matmul of wall time.
  - Each (pair, qh) sub-iteration covers a 512-wide q chunk over all 16
    kv tiles. Scores PSUM tile [128,1024] = [S_A(512) | S_B(512)] so a
    single ACT exp covers both heads.
  - exp on ACT (the only engine with exp); multiplicative 0/1 mask on
    DVE/Pool alternating; PV accumulates [65,512] per head with a
    ones-column in V giving the softmax denominator.
  - V proj, Q/K proj for later pairs, and DMAs are drip-fed into the PE
    between attention matmuls (2 closure-units per kv step).
  - PSUM (8 banks): 2x scores [128,1024] (4) + 2x PV [65,512] (2) +
    proj accum [128,512] (1) + V-proj accum [128,512] (1).

All matmuls in bf16 with fp32 PSUM accumulation.
"""

import sys

if "/opt/trn_rl_repo" not in sys.path:
    sys.path.insert(0, "/opt/trn_rl_repo")

import numpy as np
import ml_dtypes

B, S, D, H = 4, 2048, 1024, 16
DK = D // H  # 64
NCORES = 8
SQ = (B * S) // NCORES  # 1024 query rows per core
NP = H // 2  # 8 head pairs
NDT = D // 128  # 8 d-tiles
NKV = S // 128  # 16 kv tiles
BF16 = ml_dtypes.bfloat16

_CACHE = {}


def _patch_tile_drain():
    """This walrus build rejects >1 sem-wait on the CTRL (drain) struct and
    wide sem-range clears; split the Tile tail-drain's waits and chunk the
    semaphore frees."""
    import concourse.tile as tile
    import concourse.mybir as mybir
    from concourse.vector_clock import ScopedClock

    if getattr(tile.TileContext, "_drain_split_patched", False):
        return

    def _drain_and_barrier(self, tick_clock, wait_clock):
        nc = self.nc
        drain_inst = nc.sync.drain()
        wait_clock.add_sem_waits(
            drain_inst.ins, ScopedClock({None: tick_clock.global_clock})
        )
        si = drain_inst.ins.sync_info
        if si is not None and len(si.on_wait) > 1:
            waits = list(si.on_wait)
            drain_inst.ins.sync_info = mybir.SyncInfo(
                on_wait=waits[:1], on_update=list(si.on_update)
            )
            for w in waits[1:]:
                extra = nc.sync.drain()
                extra.ins.sync_info = mybir.SyncInfo(on_wait=[w], on_update=[])
        nc.all_engine_barrier()
        popped = nc._tile_sem_poison_stack.pop()
        assert popped is self._sem_poison
        sems = sorted(
            self.sems.allocated().values(),
            key=lambda s: s.num if hasattr(s, "num") else s,
        )
        for i in range(0, len(sems), 3):
            nc.clear_and_free_semaphores(sems[i : i + 3])
        nc.all_engine_barrier()

    tile.TileContext._drain_and_barrier = _drain_and_barrier
    tile.TileContext._drain_split_patched = True


def _split_excess_waits(nc, max_waits=1):
    """Walrus (this build) rejects instructions with more than one sem-wait.
    Move overflow waits onto same-engine EventSemaphore instructions inserted
    just before the overloaded instruction (per-engine order preserved)."""
    import concourse.mybir as mybir

    n = 0
    for fn in nc.m.functions:
        for bb in fn.blocks:
            out = []
            changed = False
            for inst in bb.instructions:
                si = getattr(inst, "sync_info", None)
                waits = list(si.on_wait) if si is not None else []
                if len(waits) > max_waits:
                    for w in waits[:-max_waits]:
                        n += 1
                        ev = mybir.InstEventSemaphore(
                            name=f"WSPLIT-{n}", ins=[], outs=[]
                        )
                        ev.engine = inst.engine
                        ev.sync_info = mybir.SyncInfo(on_wait=[w], on_update=[])
                        out.append(ev)
                    inst.sync_info = mybir.SyncInfo(
                        on_wait=waits[-max_waits:], on_update=list(si.on_update)
                    )
                    changed = True
                out.append(inst)
            if changed:
                bb.instructions = out
    return n


def _build():
    """Build the single-core SPMD Bass program (same for all 8 cores)."""
    import concourse.bass as bass
    import concourse.tile as tile
    import concourse.mybir as mybir

    _patch_tile_drain()

    f32 = mybir.dt.float32
    bf16 = mybir.dt.bfloat16
    ACT = mybir.ActivationFunctionType

    nc = bass.Bass("TRN2", target_bir_lowering=False, debug=False)

    # ---- kernel I/O (per-core shards, host-prepped, kv-rotated) ----
    xt_c = nc.dram_tensor("xt_c", [D, S], bf16, kind="ExternalInput").ap()
    # value seq, kv-major: [kv, 128 part, d-tile, 128 col]
    xt_v2 = nc.dram_tensor(
        "xt_v2", [NKV, 128, NDT, 128], bf16, kind="ExternalInput"
    ).ap()
    # mask^T slice, 1.0 = KEEP (multiplicative, applied post-exp)
    mskt = nc.dram_tensor("mskt", [S, SQ], bf16, kind="ExternalInput").ap()
    # wq/wk: [pair, dtile, 128 d, 128 cols(2 heads x 64 dk)]
    wq = nc.dram_tensor("wq", [NP, NDT, 128, 128], bf16, kind="ExternalInput").ap()
    wk = nc.dram_tensor("wk", [NP, NDT, 128, 128], bf16, kind="ExternalInput").ap()
    # wv: [dtile, 128 d, 1024 cols(16 heads x 64)]
    wv = nc.dram_tensor("wv", [NDT, 128, D], bf16, kind="ExternalInput").ap()
    # wot: Wo^T tiled [dtile, 128 din, 1024 dout]
    wot = nc.dram_tensor("wot", [NDT, 128, D], bf16, kind="ExternalInput").ap()
    bq_t = nc.dram_tensor("bq_t", [128, NP], f32, kind="ExternalInput").ap()
    bk_t = nc.dram_tensor("bk_t", [128, NP], f32, kind="ExternalInput").ap()
    # bv broadcast to all kv partitions, cols = h*64+c (folded into vaug:
    # sum_kv p*(V+bv) / sum_kv p == PV/sum + bv exactly)
    bv_bc = nc.dram_tensor("bv_bc", [128, D], f32, kind="ExternalInput").ap()
    bo_bc = nc.dram_tensor("bo_bc", [128, D], f32, kind="ExternalInput").ap()
    y = nc.dram_tensor("y", [SQ, D], f32, kind="ExternalOutput").ap()

    with tile.TileContext(nc) as tc:
        with (
            tc.tile_pool(name="persist", bufs=1) as persist,
            # PSUM pools (8 banks of [128, 2KB] total):
            tc.tile_pool(name="pssc", bufs=2, space="PSUM") as pssc,  # 4 banks
            tc.tile_pool(name="pspv", bufs=2, space="PSUM") as pspv,  # 2 banks
            # proj + vproj drip accumulators share one 2-slot pool so
            # back-to-back drip chunks don't serialize on the copy-out
            tc.tile_pool(name="pswk", bufs=2, space="PSUM") as pswk,  # 2 banks
        ):
            # ---- persistent small tensors ----
            bq_sb = persist.tile([128, NP], f32, tag="bq")
            nc.sync.dma_start(bq_sb[:], bq_t[:])
            bk_sb = persist.tile([128, NP], f32, tag="bk")
            nc.sync.dma_start(bk_sb[:], bk_t[:])
            bv_sb = persist.tile([128, D], f32, tag="bv")
            nc.sync.dma_start(bv_sb[:], bv_bc[:])
            bo_sb = persist.tile([128, D], f32, tag="bo")
            nc.sync.dma_start(bo_sb[:], bo_bc[:])

            # V augmented with a ones column per head: [128 kv, 2*65]
            # (col 64/129 = 1.0 -> PV matmul row 64 = softmax denominator)
            vaug_p = [
                persist.tile([128, NKV * 130], bf16, tag=f"va{p}", name=f"va{p}")
                for p in range(NP)
            ]

            def vaug(p, kv):
                return vaug_p[p][:, kv * 130 : (kv + 1) * 130]

            # concat_T: 8 din-tiles [128, SQ]
            concat = [
                persist.tile([128, SQ], bf16, tag=f"cc{p}", name=f"cc{p}")
                for p in range(NP)
            ]

            with (
                tc.tile_pool(name="xtc", bufs=1) as xtcp,
                tc.tile_pool(name="xv", bufs=3) as xvp,
                tc.tile_pool(name="wvp", bufs=1) as wvp,
                tc.tile_pool(name="wqk", bufs=2) as wqkp,
                tc.tile_pool(name="qkt", bufs=3) as qktp,
                tc.tile_pool(name="pexp", bufs=2) as pexp,
                tc.tile_pool(name="pmask", bufs=2) as pmask,
                tc.tile_pool(name="fin", bufs=2) as finp,
                tc.tile_pool(name="wot", bufs=1) as wotp,
                tc.tile_pool(name="outsb", bufs=2) as outp,
                tc.tile_pool(name="dscr", bufs=2, space="DRAM") as dscr,
            ):
                # ---- startup DMAs, priority order ----
                wq0_sb = wqkp.tile([128, NDT, 128], bf16, tag="wq", name="wq0")
                nc.sync.dma_start(wq0_sb[:], wq[0].rearrange("t d c -> d t c"))
                wk0_sb = wqkp.tile([128, NDT, 128], bf16, tag="wk", name="wk0")
                nc.sync.dma_start(wk0_sb[:], wk[0].rearrange("t d c -> d t c"))
                xtc_sb = []
                for d in range(NDT):
                    t = xtcp.tile([128, S], bf16, tag=f"xtc{d}", name=f"xtc{d}")
                    nc.sync.dma_start(t[:], xt_c[d * 128 : (d + 1) * 128, :])
                    xtc_sb.append(t)
                msk_sb = []
                for kv in range(4):
                    t = persist.tile([128, SQ], bf16, tag=f"m{kv}", name=f"m{kv}")
                    nc.sync.dma_start(t[:], mskt[kv * 128 : (kv + 1) * 128, :])
                    msk_sb.append(t)
                wv_sb = []
                for d in range(NDT):
                    t = wvp.tile([128, D], bf16, tag=f"wv{d}", name=f"wv{d}")
                    nc.sync.dma_start(t[:], wv[d, :, :])
                    wv_sb.append(t)
                for kv in range(4, NKV):
                    t = persist.tile([128, SQ], bf16, tag=f"m{kv}", name=f"m{kv}")
                    nc.sync.dma_start(t[:], mskt[kv * 128 : (kv + 1) * 128, :])
                    msk_sb.append(t)

                for p in range(NP):
                    ones_ap = vaug_p[p].rearrange("a (k c) -> a k c", c=65)[
                        :, :, 64:65
                    ]
                    nc.gpsimd.memset(ones_ap, 1.0)

                qt_all = [None] * NP
                kt_all = [None] * NP
                wqk_sb = {0: (wq0_sb, wk0_sb)}

                # ---- emitters (closure-units of ~2 matmuls each) ----
                def proj_steps(p, dma_w=True):
                    """Q/K projection for pair p. Q reads xt_c cols 0..1023
                    (the kv rotation puts this core's queries there)."""
                    state = {}

                    def dma_weights():
                        wq_sb = wqkp.tile(
                            [128, NDT, 128], bf16, tag="wq", name=f"wq{p}"
                        )
                        nc.sync.dma_start(wq_sb[:], wq[p].rearrange("t d c -> d t c"))
                        wk_sb = wqkp.tile(
                            [128, NDT, 128], bf16, tag="wk", name=f"wk{p}"
                        )
                        nc.sync.dma_start(wk_sb[:], wk[p].rearrange("t d c -> d t c"))
                        wqk_sb[p] = (wq_sb, wk_sb)

                    if dma_w:
                        yield dma_weights

                    def alloc_qt():
                        qt_all[p] = qktp.tile([128, SQ], bf16, tag="qt", name=f"qt{p}")
                        kt_all[p] = qktp.tile([128, S], bf16, tag="kt", name=f"kt{p}")

                    yield alloc_qt

                    def chunk(which, ck):
                        cs = slice(ck * 512, (ck + 1) * 512)

                        def mm(d0):
                            def f():
                                w_sb = wqk_sb[p][0 if which == "q" else 1]
                                if d0 == 0:
                                    state["ps"] = pswk.tile(
                                        [128, 512],
                                        f32,
                                        tag="wk",
                                        name=f"pj{which}{p}_{ck}",
                                    )
                                ps = state["ps"]
                                for d in (d0, d0 + 1):
                                    nc.tensor.matmul(
                                        ps[:],
                                        w_sb[:, d, :],
                                        xtc_sb[d][:, cs],
                                        start=(d == 0),
                                        stop=(d == NDT - 1),
                                    )
                                if d0 == NDT - 2:
                                    dst = qt_all[p] if which == "q" else kt_all[p]
                                    bias = bq_sb if which == "q" else bk_sb
                                    # gpsimd cannot read PSUM; bias-add on DVE
                                    nc.vector.tensor_scalar_add(
                                        dst[:, cs], ps[:], bias[:, p : p + 1]
                                    )

                            return f

                        return [mm(d0) for d0 in range(0, NDT, 2)]

                    for ck in range(2):  # Q: q cols 0..1023
                        yield from chunk("q", ck)
                    for ck in range(4):  # K: all of S
                        yield from chunk("k", ck)

                def vproj_steps(kv, ch):
                    """V projection for (kv tile, 512-col chunk ch): DMA x
                    chunk, 8 matmuls into [128,512], 4 Pool copies to vaug."""
                    state = {}

                    def dma_xv():
                        t = xvp.tile(
                            [128, NDT, 128], bf16, tag="xv", name=f"xv{kv}_{ch}"
                        )
                        nc.sync.dma_start(t[:], xt_v2[kv])
                        state["xv"] = t

                    yield dma_xv

                    def v_mm(d0):
                        def f():
                            if d0 == 0:
                                state["ps"] = pswk.tile(
                                    [128, 512], f32, tag="wk", name=f"psv{kv}_{ch}"
                                )
                            ps = state["ps"]
                            for d in range(d0, d0 + 2):
                                nc.tensor.matmul(
                                    ps[:],
                                    state["xv"][:, d, :],
                                    wv_sb[d][:, ch * 512 : (ch + 1) * 512],
                                    start=(d == 0),
                                    stop=(d == NDT - 1),
                                )

                        return f

                    for d0 in range(0, NDT, 2):
                        yield v_mm(d0)

                    def v_copy():
                        ps = state["ps"]
                        for pi in range(4):
                            p = ch * 4 + pi
                            dst = vaug(p, kv).rearrange("a (h c) -> a h c", c=65)[
                                :, :, 0:64
                            ]
                            src = ps[:, pi * 128 : (pi + 1) * 128].rearrange(
                                "a (h c) -> a h c", c=64
                            )
                            bvs = bv_sb[:, p * 128 : (p + 1) * 128].rearrange(
                                "a (h c) -> a h c", c=64
                            )
                            nc.vector.tensor_add(dst, src, bvs)

                    yield v_copy

                from collections import deque

                work = deque()

                def drain(n):
                    for _ in range(n):
                        if work:
                            work.popleft()()

                # ---- upfront: Q/K proj pair 0, V proj ch0 for kv 0..13 ----
                for step in proj_steps(0, dma_w=False):
                    step()
                for kv in range(14):
                    for step in vproj_steps(kv, 0):
                        step()

                # drip queue, deadline-ordered; proj p+2 is enqueued at
                # pair-p start (its qkt pool slot frees exactly then)
                for kv in (14, 15):
                    work.extend(vproj_steps(kv, 0))
                work.extend(proj_steps(1))
                for kv in range(8):
                    work.extend(vproj_steps(kv, 1))
                work.extend(proj_steps(2))
                for kv in range(8, NKV):
                    work.extend(vproj_steps(kv, 1))

                wot_sb = []

                def dma_wot():
                    for d in range(NDT):
                        t = wotp.tile([128, D], bf16, tag=f"wot{d}", name=f"wot{d}")
                        nc.sync.dma_start(t[:], wot[d, :, :])
                        wot_sb.append(t)

                # ---- attention: pair p, q-half qh, kv 0..15 ----
                def fin_head(p, h, qh, ps_o):
                    """Normalize PV accum [65,512] -> concat; reciprocal of
                    the denominator via DRAM bounce into all DVE lanes."""
                    qs = slice(qh * 512, (qh + 1) * 512)
                    o_sb = finp.tile([65, 512], f32, tag="osb")
                    nc.vector.tensor_copy(o_sb[:], ps_o[:])
                    dsum = dscr.tile([512], f32, tag="dsum")
                    nc.sync.dma_start(
                        dsum.rearrange("(a b) -> a b", a=1), o_sb[64:65, :]
                    )
                    rs = finp.tile([128, 4], f32, tag="rs")
                    nc.sync.dma_start(rs[:], dsum.rearrange("(a b) -> a b", a=128))
                    rr = finp.tile([128, 4], f32, tag="rr")
                    nc.vector.reciprocal(rr[:], rs[:])
                    drec = dscr.tile([512], f32, tag="drec")
                    nc.sync.dma_start(drec.rearrange("(a b) -> a b", a=128), rr[:])
                    rb = finp.tile([64, 512], f32, tag="rb")
                    nc.sync.dma_start(
                        rb[:],
                        drec.rearrange("(a b) -> a b", a=1).partition_broadcast(64),
                    )
                    nc.vector.tensor_mul(
                        concat[p][h * 64 : (h + 1) * 64, qs], rb[:], o_sb[0:64, :]
                    )

                def outproj_steps(qt_i, ch):
                    """Output projection chunk [128 q, 512 dout] as 2-MM
                    units + a finish unit (DVE bias-add, DMA out)."""
                    state = {}
                    qs = slice(qt_i * 128, (qt_i + 1) * 128)
                    chs = slice(ch * 512, (ch + 1) * 512)

                    def mm(d0):
                        def f():
                            if d0 == 0:
                                state["ps"] = pswk.tile(
                                    [128, 512], f32, tag="wk", name=f"of{qt_i}_{ch}"
                                )
                            ps = state["ps"]
                            for d in (d0, d0 + 1):
                                nc.tensor.matmul(
                                    ps[:],
                                    concat[d][:, qs],
                                    wot_sb[d][:, chs],
                                    start=(d == 0),
                                    stop=(d == NDT - 1),
                                )

                        return f

                    for d0 in range(0, NDT, 2):
                        yield mm(d0)

                    def finish():
                        ps = state["ps"]
                        out_sb = outp.tile([128, 512], f32, tag="out")
                        nc.vector.tensor_add(out_sb[:], ps[:], bo_sb[:, chs])
                        nc.sync.dma_start(y[qs, chs], out_sb[:])

                    yield finish

                pending_fin = deque()
                for p in range(NP):
                    if 1 <= p and p + 2 < NP:
                        work.extend(proj_steps(p + 2))
                    if p == 5:
                        work.append(dma_wot)
                    qt, kt = qt_all[p], kt_all[p]
                    for qh in range(2):
                        if p == NP - 1 and qh == 1:
                            # overlap out-proj for the finished qh0 half
                            for qt_i in range(4):
                                for ch in range(2):
                                    work.extend(outproj_steps(qt_i, ch))
                        qs = slice(qh * 512, (qh + 1) * 512)
                        ps_oA = pspv.tile([65, 512], f32, tag="po", name=f"poA{p}_{qh}")
                        ps_oB = pspv.tile([65, 512], f32, tag="po", name=f"poB{p}_{qh}")
                        prev_pm = None
                        for kv in range(NKV):
                            kvs = slice(kv * 128, (kv + 1) * 128)
                            ps_s = pssc.tile(
                                [128, 1024], f32, tag="ps", name=f"s{p}_{qh}_{kv}"
                            )
                            # row-tiled scores: head A rows 0-63, B rows 64-127
                            nc.tensor.matmul(
                                ps_s[:, 0:512],
                                kt[0:64, kvs],
                                qt[0:64, qs],
                                start=True,
                                stop=True,
                            )
                            nc.tensor.matmul(
                                ps_s[:, 512:1024],
                                kt[64:128, kvs],
                                qt[64:128, qs],
                                start=True,
                                stop=True,
                            )
                            if prev_pm is not None:
                                # PV for kv-1 (both heads), one behind
                                nc.tensor.matmul(
                                    ps_oA[:],
                                    vaug(p, kv - 1)[:, 0:65],
                                    prev_pm[:, 0:512],
                                    start=(kv == 1),
                                    stop=False,
                                )
                                nc.tensor.matmul(
                                    ps_oB[:],
                                    vaug(p, kv - 1)[:, 65:130],
                                    prev_pm[:, 512:1024],
                                    start=(kv == 1),
                                    stop=False,
                                )
                            drain(2 if p < 3 else 1)
                            if pending_fin and kv == 1:
                                fin_head(*pending_fin.popleft())
                            if pending_fin and kv == 2:
                                fin_head(*pending_fin.popleft())
                            pe_t = pexp.tile([128, 1024], bf16, tag="pe")
                            nc.scalar.activation(
                                pe_t[:], ps_s[:], ACT.Exp, scale=0.125
                            )
                            pm_t = pmask.tile([128, 1024], bf16, tag="pm")
                            mb = (
                                msk_sb[kv][:, qs]
                                .rearrange("p (o c) -> p o c", o=1)
                                .broadcast_to([128, 2, 512])
                            )
                            nc.vector.tensor_mul(
                                pm_t.rearrange("p (o c) -> p o c", c=512),
                                pe_t.rearrange("p (o c) -> p o c", c=512),
                                mb,
                            )
                            prev_pm = pm_t
                        # last PV (kv = 15)
                        nc.tensor.matmul(
                            ps_oA[:],
                            vaug(p, NKV - 1)[:, 0:65],
                            prev_pm[:, 0:512],
                            start=False,
                            stop=True,
                        )
                        nc.tensor.matmul(
                            ps_oB[:],
                            vaug(p, NKV - 1)[:, 65:130],
                            prev_pm[:, 512:1024],
                            start=False,
                            stop=True,
                        )
                        pending_fin.append((p, 0, qh, ps_oA))
                        pending_fin.append((p, 1, qh, ps_oB))
                while pending_fin:
                    fin_head(*pending_fin.popleft())

                # ---- output projection (qh1 half + any undrained qh0) ----
                for qt_i in range(4, SQ // 128):
                    for ch in range(2):
                        work.extend(outproj_steps(qt_i, ch))
                while work:
                    work.popleft()()

    _split_excess_waits(nc, max_waits=1)
    return nc


def _prep_inputs(context_sequence, value_sequence, mask, Wq, bq, Wk, bk, Wv, bv, Wo, bo):
    """Host-side shard prep: slice/transpose/cast per core, with the kv
    axis rotated by sh*SQ so queries are at columns 0..SQ-1."""
    ctx = np.asarray(context_sequence, dtype=np.float32)
    val = np.asarray(value_sequence, dtype=np.float32)
    mask = np.asarray(mask)
    Wq = np.asarray(Wq, dtype=np.float32)
    Wk = np.asarray(Wk, dtype=np.float32)
    Wv = np.asarray(Wv, dtype=np.float32)
    Wo = np.asarray(Wo, dtype=np.float32)
    bq = np.asarray(bq, dtype=np.float32)
    bk = np.asarray(bk, dtype=np.float32)
    bv = np.asarray(bv, dtype=np.float32)
    bo = np.asarray(bo, dtype=np.float32)

    def wtile(W):  # [H, D, DK] -> [NP, NDT, 128, 128]
        Wf = W.transpose(1, 0, 2).reshape(D, D)  # [d, h*dk]
        return np.ascontiguousarray(
            Wf.reshape(NDT, 128, NP, 128).transpose(2, 0, 1, 3)
        ).astype(BF16)

    wq_t = wtile(Wq)
    wk_t = wtile(Wk)
    wv_t = np.ascontiguousarray(
        Wv.transpose(1, 0, 2).reshape(D, D).reshape(NDT, 128, D)
    ).astype(BF16)
    wot_t = np.ascontiguousarray(Wo.T.reshape(NDT, 128, D)).astype(BF16)
    bq_t = np.ascontiguousarray(bq.reshape(NP, 128).T)  # [128, NP]
    bk_t = np.ascontiguousarray(bk.reshape(NP, 128).T)
    bv_bc = np.ascontiguousarray(
        np.broadcast_to(bv.reshape(D)[None, :], (128, D))
    )  # [128, D], cols h*64+c
    bo_bc = np.ascontiguousarray(np.broadcast_to(bo[None, :], (128, D)))

    in_maps = []
    for c in range(NCORES):
        b, sh = c // 2, c % 2
        roll = -sh * SQ  # rotate kv so this core's queries sit at cols 0..SQ-1
        xt = np.ascontiguousarray(
            np.roll(ctx[b].T, roll, axis=1)
        ).astype(BF16)  # [D, S], kv-rotated
        xtv = np.roll(val[b].T, roll, axis=1).astype(BF16)  # [D, S]
        xtv2 = np.ascontiguousarray(
            xtv.reshape(NDT, 128, NKV, 128).transpose(2, 1, 0, 3)
        )  # [kv, p, d, c]
        mskt = np.ascontiguousarray(
            np.roll(
                (mask[sh * SQ : (sh + 1) * SQ, :] == 0).T.astype(BF16),
                roll,
                axis=0,
            )
        )  # [S, SQ], 1.0 = keep, kv-rotated
        in_maps.append(
            {
                "xt_c": xt,
                "xt_v2": xtv2,
                "mskt": mskt,
                "wq": wq_t,
                "wk": wk_t,
                "wv": wv_t,
                "wot": wot_t,
                "bq_t": bq_t,
                "bk_t": bk_t,
                "bv_bc": bv_bc,
                "bo_bc": bo_bc,
            }
        )
    return in_maps


def _execute(inputs, trace=False):
    from concourse.bass_utils import run_bass_kernel_spmd

    if "nc" not in _CACHE:
        _CACHE["nc"] = _build()
    nc = _CACHE["nc"]
    in_maps = _prep_inputs(**inputs)
    res = run_bass_kernel_spmd(nc, in_maps, list(range(NCORES)), trace=trace)
    out = np.empty((B, S, D), dtype=np.float32)
    for c in range(NCORES):
        b, sh = c // 2, c % 2
        out[b, sh * SQ : (sh + 1) * SQ, :] = res.results[c]["y"]
    return out, res.exec_time_ns


def kernel(**inputs):
    out, _ = _execute(inputs, trace=False)
    return out
